# revision 3
# baseline (speedup 1.0000x reference)
"""Two-layer single-head GAT (GATConv x2) on 8 trn2 NeuronCores.

Strategy: 1D node partition across 8 cores by destination node; edges live
with their destination owner, so edge-softmax and the scatter-aggregate stay
local. Weights replicated. Both layers use the SAME pi-order table row
mapping (row = PI0 + core*NSHP + pos), so the graph plan, slot layout and
gather-index tensor are built once and shared by both launches.

Per layer, per core:
  Stage A (dense, PE, bf16): table rows T = h @ (W * a_src) written to a DRAM
    gather table (f32), 8 chunks per PSUM bank, batched 1024-row DMA writes.
    ad = h_own @ (W @ a_dst) for the core's own destinations.
  Stage B (sparse): destinations are degree-sorted, processed in tiles of 128
    (one per SBUF partition). Tiles are batched into gather GROUPS to
    amortize the ~1us fixed SWDGE cost; each group does one "lo" gather
    (table rows < 32768) and one "hi" gather (rows >= HI_BASE) because
    dma_gather indices are int16. Per tile: as = rowsum(T_gathered),
    s = lrelu(as + ad), p = exp(s) with fused accumulated denominator, then
    U = sum_k p_k * T_k via a ping-pong scalar_tensor_tensor chain, and
    out = U * (1/a_src) / den + b.
  Padded slots point at PAD rows filled with -1e30 => p == 0 exactly.

The bottleneck is SWDGE descriptor generation on gpsimd (~8ns/row); per-core
slot counts are within ~3% of the per-core edge count, which is the floor.
"""

import sys

sys.path.insert(0, "/opt/trn_rl_repo")

import numpy as np

N = 50000
E = 800000
IN = 128
OUT = 64
C = 8                       # cores
NSH = N // C                # 6250 dsts per core
NTILES = (NSH + 127) // 128  # 49
NSHP = NTILES * 128         # 6272 padded dsts per core
NEG_SLOPE = 0.2

PI0 = 128                   # first pi row (chunk 0 = pad chunk)
ROWS = PI0 + C * NSHP + 128  # 50432: pad chunk + pi rows + hi-pad chunk
NCHUNK = ROWS // 128        # 394
LO_MAX_ROW = 32767
HI_BASE = ROWS - 32768      # 17664; hi window covers [HI_BASE, ROWS)
PAD_VAL = -1.0e30
PADROW_LO = 0
PADROW_HI = ROWS - 1        # 50431 (inside the trailing pad chunk)

GROUP_SLOT_BUDGET = 96      # max sum of (K_lo+K_hi) per gather group


def _build_plan(edge_index):
    """Host-side graph preprocessing: slot layout shared by both layers."""
    src = np.concatenate([np.asarray(edge_index[0], dtype=np.int64), np.arange(N)])
    dst = np.concatenate([np.asarray(edge_index[1], dtype=np.int64), np.arange(N)])

    core_of = dst // NSH
    orders = []          # per core: global node id at each sorted position
    pos_of = np.empty(N, dtype=np.int64)
    for c in range(C):
        d0 = c * NSH
        deg_c = np.bincount(dst[core_of == c] - d0, minlength=NSH)
        order = np.argsort(-deg_c, kind="stable")
        pos_of[d0 + order] = np.arange(NSH)
        orders.append(np.concatenate([order + d0, np.full(NSHP - NSH, -1, np.int64)]))

    rowmap = PI0 + (np.arange(N) // NSH) * NSHP + pos_of   # node -> table row
    row_s = rowmap[src]
    lo_elig = row_s <= LO_MAX_ROW
    hi_elig = row_s >= HI_BASE
    lo_only = lo_elig & ~hi_elig
    hi_only = hi_elig & ~lo_elig

    epos = pos_of[dst]
    Klo_ct = np.zeros((C, NTILES), np.int64)
    Khi_ct = np.zeros((C, NTILES), np.int64)
    for c in range(C):
        m = core_of == c
        deg_p = np.bincount(epos[m], minlength=NSHP)
        lo_p = np.bincount(epos[m & lo_only], minlength=NSHP)
        hi_p = np.bincount(epos[m & hi_only], minlength=NSHP)
        deg_t = deg_p.reshape(NTILES, 128).max(1)
        mlo_t = lo_p.reshape(NTILES, 128).max(1)
        mhi_t = hi_p.reshape(NTILES, 128).max(1)
        Klo_ct[c] = mlo_t
        Khi_ct[c] = np.maximum(mhi_t, deg_t - mlo_t)
    K_lo = Klo_ct.max(0)
    K_hi = Khi_ct.max(0)

    # per-edge slot assignment: rank edges within each (core, pos):
    # hi_only first, then flex, then lo_only
    cat = np.where(hi_only, 0, np.where(lo_only, 2, 1))
    okey = np.lexsort((cat, epos, core_of))
    sc, pc, catc, rsc = core_of[okey], epos[okey], cat[okey], row_s[okey]
    gid = sc * NSHP + pc
    first = np.r_[True, gid[1:] != gid[:-1]]
    idx_lin = np.arange(len(gid))
    start = np.maximum.accumulate(np.where(first, idx_lin, 0))
    rank = idx_lin - start
    hi_only_cnt = np.bincount(gid[catc == 0], minlength=C * NSHP)
    flex_cnt = np.bincount(gid[catc == 1], minlength=C * NSHP)
    hi_target = np.minimum(hi_only_cnt + flex_cnt,
                           K_hi[(np.arange(C * NSHP) % NSHP) // 128])
    ht_e = hi_target[gid]
    is_hi = rank < ht_e
    slot_hi = rank
    slot_lo = rank - ht_e
    assert (rsc[is_hi] >= HI_BASE).all()
    assert (rsc[~is_hi] <= LO_MAX_ROW).all()
    assert (slot_lo[~is_hi] < K_lo[(pc[~is_hi]) // 128]).all()

    plan = dict(K_lo=K_lo, K_hi=K_hi, core=sc, pos=pc, row=rsc,
                is_hi=is_hi, slot=np.where(is_hi, slot_hi, slot_lo))

    # gather groups: consecutive tiles, sum of (K_lo+K_hi) <= budget
    groups = []
    cur = []
    acc = 0
    for t in range(NTILES):
        k = int(K_lo[t] + K_hi[t])
        if cur and acc + k > GROUP_SLOT_BUDGET:
            groups.append(cur)
            cur, acc = [], 0
        cur.append(t)
        acc += k
    if cur:
        groups.append(cur)

    return orders, pos_of, rowmap, plan, groups


def _wrap_idx(arr):
    """[K,128] slot-major idx array -> [128, 8K] wrapped+replicated int16."""
    flat = arr.reshape(-1)                       # i = k*128 + p
    w = flat.reshape(-1, 16).T                   # [16, NI/16]
    return np.tile(w, (8, 1)).astype(np.int16)


def _build_idx_tensor(plan, groups):
    """Per-core [128, IDXCOLS] int16 idx tensor; group/tile offset tables.

    Layout per group: [all lo slots of tiles in group][all hi slots].
    Returns (idx[C,128,idxcols], ginfo) where ginfo[g] =
    (olo, KLg, ohi, KHg, tiles=[(t, lo_off, kl, hi_off, kh), ...]).
    """
    K_lo, K_hi = plan["K_lo"], plan["K_hi"]
    ginfo = []
    off = 0
    for g in groups:
        KLg = int(sum(K_lo[t] for t in g))
        KHg = int(sum(K_hi[t] for t in g))
        olo = off
        ohi = off + 8 * KLg
        off = ohi + 8 * KHg
        tiles = []
        lo_off = hi_off = 0
        for t in g:
            tiles.append((t, lo_off, int(K_lo[t]), hi_off, int(K_hi[t])))
            lo_off += int(K_lo[t])
            hi_off += int(K_hi[t])
        ginfo.append((olo, KLg, ohi, KHg, tiles))
    idxcols = off

    out = np.zeros((C, 128, idxcols), np.int16)
    pad_hi = PADROW_HI - HI_BASE
    core_a, pos_a = plan["core"], plan["pos"]
    row_a, ishi_a, slot_a = plan["row"], plan["is_hi"], plan["slot"]
    for c in range(C):
        m = core_a == c
        pos, row, is_hi, slot = pos_a[m], row_a[m], ishi_a[m], slot_a[m]
        tile = pos // 128
        part = pos % 128
        for (olo, KLg, ohi, KHg, tiles) in ginfo:
            lo_arr = np.zeros((KLg, 128), np.int64)            # pad -> row 0
            hi_arr = np.full((KHg, 128), pad_hi, np.int64)
            for (t, lo_off, kl, hi_off, kh) in tiles:
                tm = tile == t
                lm = tm & ~is_hi
                hm = tm & is_hi
                lo_arr[lo_off + slot[lm], part[lm]] = row[lm]
                hi_arr[hi_off + slot[hm], part[hm]] = row[hm] - HI_BASE
            if KLg:
                out[c, :, olo:olo + 8 * KLg] = _wrap_idx(lo_arr)
            if KHg:
                out[c, :, ohi:ohi + 8 * KHg] = _wrap_idx(hi_arr)
    return out, ginfo, idxcols


def _build_launch(kdim, ginfo, idxcols):
    """One SPMD launch: Stage A (table build) + Stage B (gather groups)."""
    import concourse.bacc as bacc
    import concourse.mybir as mybir
    from concourse.tile import TileContext

    f32 = mybir.dt.float32
    bf16 = mybir.dt.bfloat16

    nc = bacc.Bacc(None, target_bir_lowering=False, debug=True)
    hT = nc.declare_dram_parameter("hT", [kdim, ROWS], bf16, isOutput=False)
    hoT = nc.declare_dram_parameter("hoT", [kdim, NSHP], bf16, isOutput=False)
    wse = nc.declare_dram_parameter("wse", [kdim, 65], bf16, isOutput=False)
    rb = nc.declare_dram_parameter("rb", [128, 128], f32, isOutput=False)
    idx = nc.declare_dram_parameter("idx", [128, idxcols], mybir.dt.int16,
                                    isOutput=False)
    outp = nc.declare_dram_parameter("outp", [NSHP, 64], f32, isOutput=True)
    tabl = nc.dram_tensor("tabl", [ROWS, 64], f32)

    SC = 8                       # chunks per super-chunk (one PSUM bank)
    n_super = NCHUNK // SC       # 49 full super-chunks
    tail = NCHUNK - n_super * SC  # 2 tail chunks

    with TileContext(nc) as tc:
        with (
            tc.tile_pool(name="const", bufs=1) as cpool,
            tc.tile_pool(name="xin", bufs=3) as xin,
            tc.tile_pool(name="stage", bufs=3) as stage,
            tc.tile_pool(name="psA", bufs=3, space="PSUM") as psA,
            tc.tile_pool(name="psB", bufs=2, space="PSUM") as psB,
            tc.tile_pool(name="tg", bufs=2) as tgp,
            tc.tile_pool(name="sm", bufs=3) as sm,
        ):
            wse_sb = cpool.tile([kdim, 65], bf16)
            nc.sync.dma_start(out=wse_sb[:, :], in_=wse[:, :])
            rb_sb = cpool.tile([128, 128], f32)
            nc.sync.dma_start(out=rb_sb[:, :], in_=rb[:, :])
            idx_sb = cpool.tile([128, idxcols], mybir.dt.int16)
            nc.sync.dma_start(out=idx_sb[:, :], in_=idx[:, :])
            ho_sb = cpool.tile([kdim, NSHP], bf16)
            nc.sync.dma_start(out=ho_sb[:, :], in_=hoT[:, :])
            ad_sb = cpool.tile([128, NTILES], f32)
            padrow = cpool.tile([128, 64], f32)
            nc.vector.memset(padrow[:, :], PAD_VAL)

            # Stage A: table rows = hT chunk . wse[:, 0:64], 8 chunks/PSUM bank
            def do_super(sc_i, nch):
                cols = 128 * nch
                xt = xin.tile([kdim, 1024], bf16, tag="xt")
                nc.sync.dma_start(out=xt[:, 0:cols],
                                  in_=hT[:, 1024 * sc_i:1024 * sc_i + cols])
                ps = psA.tile([128, 512], f32, tag="ps")
                for j in range(nch):
                    nc.tensor.matmul(ps[:, 64 * j:64 * (j + 1)],
                                     xt[:, 128 * j:128 * (j + 1)],
                                     wse_sb[:, 0:64], start=True, stop=True)
                st = stage.tile([128, 512], f32, tag="st")
                nc.vector.tensor_copy(st[:, 0:64 * nch], ps[:, 0:64 * nch])
                # st[p, j*64+f] -> tabl row (sc_i*1024 + j*128 + p), col f
                dst = tabl[1024 * sc_i:1024 * sc_i + cols, :] \
                    .rearrange("(c p) f -> p c f", p=128)
                nc.sync.dma_start(out=dst, in_=st[:, 0:64 * nch]
                                  .rearrange("p (c f) -> p c f", f=64))

            for sc_i in range(n_super):
                do_super(sc_i, SC)
            if tail:
                do_super(n_super, tail)

            # pad rows AFTER stage A writes
            nc.sync.dma_start(out=tabl[PADROW_LO:PADROW_LO + 1, :],
                              in_=padrow[0:1, :])
            nc.sync.dma_start(out=tabl[PADROW_HI:PADROW_HI + 1, :],
                              in_=padrow[0:1, :])

            # ad for own (sorted) dsts: ho chunk . wse[:, 64]
            for t in range(NTILES):
                ps2 = psB.tile([128, 1], f32, tag="ps2")
                nc.tensor.matmul(ps2[:, :], ho_sb[:, 128 * t:128 * (t + 1)],
                                 wse_sb[:, 64:65], start=True, stop=True)
                nc.scalar.copy(ad_sb[:, t:t + 1], ps2[:, :])

            # Stage B: gather groups
            for (olo, KLg, ohi, KHg, tiles) in ginfo:
                tg_lo = tgp.tile([128, max(KLg, 1), 64], f32, tag="tglo")
                tg_hi = tgp.tile([128, max(KHg, 1), 64], f32, tag="tghi")
                if KLg:
                    nc.gpsimd.dma_gather(tg_lo[:, 0:KLg, :],
                                         tabl[0:LO_MAX_ROW + 1, :],
                                         idx_sb[:, olo:olo + 8 * KLg],
                                         128 * KLg, 128 * KLg, 64,
                                         single_packet=False)
                if KHg:
                    nc.gpsimd.dma_gather(tg_hi[:, 0:KHg, :],
                                         tabl[HI_BASE:ROWS, :],
                                         idx_sb[:, ohi:ohi + 8 * KHg],
                                         128 * KHg, 128 * KHg, 64,
                                         single_packet=False)
                for (t, lo_off, kl, hi_off, kh) in tiles:
                    k = kl + kh
                    as_t = sm.tile([128, k], f32, tag="as")
                    if kl:
                        nc.vector.tensor_reduce(as_t[:, 0:kl],
                                                tg_lo[:, lo_off:lo_off + kl, :],
                                                mybir.AxisListType.X,
                                                mybir.AluOpType.add)
                    if kh:
                        nc.vector.tensor_reduce(as_t[:, kl:k],
                                                tg_hi[:, hi_off:hi_off + kh, :],
                                                mybir.AxisListType.X,
                                                mybir.AluOpType.add)
                    z_t = sm.tile([128, k], f32, tag="z")
                    nc.vector.tensor_scalar(z_t[:, :], as_t[:, :],
                                            ad_sb[:, t:t + 1], None,
                                            mybir.AluOpType.add)
                    s_t = sm.tile([128, k], f32, tag="s")
                    nc.vector.scalar_tensor_tensor(s_t[:, :], z_t[:, :],
                                                   NEG_SLOPE, z_t[:, :],
                                                   mybir.AluOpType.mult,
                                                   mybir.AluOpType.max)
                    p_t = sm.tile([128, k], f32, tag="p")
                    den = sm.tile([128, 1], f32, tag="den")
                    nc.scalar.activation(p_t[:, :], s_t[:, :],
                                         mybir.ActivationFunctionType.Exp,
                                         accum_out=den[:, :])

                    def slot_src(j):
                        if j < kl:
                            return tg_lo[:, lo_off + j, :]
                        return tg_hi[:, hi_off + (j - kl), :]

                    u0 = sm.tile([128, 64], f32, tag="u0")
                    u1 = sm.tile([128, 64], f32, tag="u1")
                    nc.vector.tensor_scalar(u0[:, :], slot_src(0),
                                            p_t[:, 0:1], None,
                                            mybir.AluOpType.mult)
                    cur, nxt = u0, u1
                    for j in range(1, k):
                        nc.vector.scalar_tensor_tensor(nxt[:, :], slot_src(j),
                                                       p_t[:, j:j + 1],
                                                       cur[:, :],
                                                       mybir.AluOpType.mult,
                                                       mybir.AluOpType.add)
                        cur, nxt = nxt, cur
                    rd = sm.tile([128, 1], f32, tag="rd")
                    nc.vector.reciprocal(rd[:, :], den[:, :])
                    o1 = sm.tile([128, 64], f32, tag="o1")
                    nc.vector.scalar_tensor_tensor(o1[:, :], cur[:, :],
                                                   rd[:, :], rb_sb[:, 0:64],
                                                   mybir.AluOpType.mult,
                                                   mybir.AluOpType.mult)
                    o2 = sm.tile([128, 64], f32, tag="o2")
                    nc.vector.tensor_tensor(o2[:, :], o1[:, :],
                                            rb_sb[:, 64:128],
                                            mybir.AluOpType.add)
                    nc.sync.dma_start(out=outp[128 * t:128 * (t + 1), :],
                                      in_=o2[:, :])

    nc.compile()
    return nc


LAST = {}


def kernel(x, edge_index, W1, a_src1, a_dst1, b1, W2, a_src2, a_dst2, b2):
    from concourse.bass_utils import run_bass_kernel_spmd
    import ml_dtypes

    bf = np.dtype(ml_dtypes.bfloat16)

    x = np.asarray(x, np.float32)
    edge_index = np.asarray(edge_index)
    W1 = np.asarray(W1, np.float32); a_src1 = np.asarray(a_src1, np.float32)
    a_dst1 = np.asarray(a_dst1, np.float32); b1 = np.asarray(b1, np.float32)
    W2 = np.asarray(W2, np.float32); a_src2 = np.asarray(a_src2, np.float32)
    a_dst2 = np.asarray(a_dst2, np.float32); b2 = np.asarray(b2, np.float32)

    orders, pos_of, rowmap, plan, groups = _build_plan(edge_index)
    idx, ginfo, idxcols = _build_idx_tensor(plan, groups)

    nc1 = _build_launch(IN, ginfo, idxcols)
    nc2 = _build_launch(OUT, ginfo, idxcols)

    def guard(a):
        return np.where(a == 0, np.float32(1e-30), a)

    w1se = np.concatenate([W1 * a_src1[None, :], (W1 @ a_dst1)[:, None]], 1).astype(bf)
    w2se = np.concatenate([W2 * a_src2[None, :], (W2 @ a_dst2)[:, None]], 1).astype(bf)
    rb1 = np.concatenate([np.tile(1.0 / guard(a_src1), (128, 1)),
                          np.tile(b1, (128, 1))], 1).astype(np.float32)
    rb2 = np.concatenate([np.tile(1.0 / guard(a_src2), (128, 1)),
                          np.tile(b2, (128, 1))], 1).astype(np.float32)

    # launch 1 inputs: hT1 in pi order
    hpi1 = np.zeros((ROWS, IN), np.float32)
    for c in range(C):
        own = orders[c]
        real = own >= 0
        hpi1[PI0 + c * NSHP + np.arange(NSHP)[real]] = x[own[real]]
    hT1 = np.ascontiguousarray(hpi1.T).astype(bf)       # [IN, ROWS]
    in_maps1 = []
    for c in range(C):
        hoT = np.ascontiguousarray(hT1[:, PI0 + c * NSHP:PI0 + (c + 1) * NSHP])
        in_maps1.append({"hT": hT1, "hoT": hoT, "wse": w1se, "rb": rb1,
                        "idx": idx[c]})

    res1 = run_bass_kernel_spmd(nc1, in_maps1, core_ids=list(range(C)))
    LAST["res1"] = res1
    shards1 = [np.asarray(res1.results[c]["outp"]) for c in range(C)]

    # assemble full h2 in pi order; zero the dummy rows
    hpi2 = np.zeros((ROWS, OUT), np.float32)
    for c in range(C):
        sh = shards1[c].copy()
        sh[orders[c] < 0] = 0.0
        hpi2[PI0 + c * NSHP:PI0 + (c + 1) * NSHP] = sh
    hT2 = np.ascontiguousarray(hpi2.T).astype(bf)       # [64, ROWS]

    in_maps2 = []
    for c in range(C):
        hoT2 = np.ascontiguousarray(hT2[:, PI0 + c * NSHP:PI0 + (c + 1) * NSHP])
        in_maps2.append({"hT": hT2, "hoT": hoT2, "wse": w2se, "rb": rb2,
                        "idx": idx[c]})

    res2 = run_bass_kernel_spmd(nc2, in_maps2, core_ids=list(range(C)))
    LAST["res2"] = res2

    out = np.empty((N, OUT), np.float32)
    for c in range(C):
        sh = np.asarray(res2.results[c]["outp"])
        own = orders[c]
        real = own >= 0
        out[own[real]] = sh[real]
    return out


# revision 5
# speedup vs baseline: 1.3954x; 1.3954x over previous
"""Two-layer single-head GAT (GATConv x2) on 8 trn2 NeuronCores.

Strategy: 1D node partition across 8 cores by destination node; edges live
with their destination owner, so edge-softmax and the scatter-aggregate stay
local. Weights replicated. Both layers share one graph plan / gather-index
tensor.

The bottleneck is SWDGE descriptor generation on gpsimd (~8ns/gathered row),
so the design minimizes gathered slots. dma_gather indices are int16, which
only reaches 32767 rows. Instead of splitting each destination's edges
across two gather windows (costs ~40% extra padded slots), each core's 49
destination tiles are split round-robin into 3 SUB-SHARDS; each sub-shard
gets its own COMPACT gather table holding just its distinct source nodes
(~27k rows < 32767), renumbered densely. Slot padding is then only the
per-tile max-degree padding (~2-3%).

Per layer, per core:
  Stage A (dense, PE, bf16): table rows T = h_src @ (W * a_src) written to
    the 3 concatenated sub-tables in DRAM (f32); host supplies hT with
    columns pre-arranged in sub-table order. ad = h_own @ (W @ a_dst).
    8 chunks per PSUM bank, batched 1024-row DMA writes.
  Stage B (sparse): destination tiles are degree-sorted, 128 dsts per tile
    (one per SBUF partition); tiles batched into gather GROUPS (sum K <= 96)
    within a sub-shard, ONE dma_gather per group. Per tile:
      as = rowsum(T_gathered)        (DVE reduce)
      s  = Lrelu(as + ad)            (Scalar activation, bias=ad)
      p  = Exp(s), den accumulated   (Scalar activation)
      rd = 1/den                     (DVE, batched per group)
      U  = sum_k p_k T_k             (DVE mult + transposed reduce)
      out = U * (1/a_src) * rd + b
  Padded slots point at each sub-table's row 0, filled with -1e30 => p == 0.
"""

import sys

sys.path.insert(0, "/opt/trn_rl_repo")

import numpy as np

N = 50000
E = 800000
IN = 128
OUT = 64
C = 8                       # cores
NSH = N // C                # 6250 dsts per core
NTILES = (NSH + 127) // 128  # 49
NSHP = NTILES * 128         # 6272 padded dsts per core
NEG_SLOPE = 0.2
NSUB = 3                    # sub-shards per core (tile t -> sub t % NSUB)
PAD_VAL = -1.0e30
GROUP_SLOT_BUDGET = 96      # max sum of K per gather group


def _build_plan(edge_index):
    """Host-side graph preprocessing shared by both layers."""
    src = np.concatenate([np.asarray(edge_index[0], dtype=np.int64), np.arange(N)])
    dst = np.concatenate([np.asarray(edge_index[1], dtype=np.int64), np.arange(N)])

    core_of = dst // NSH
    orders = []
    pos_of = np.empty(N, dtype=np.int64)
    for c in range(C):
        d0 = c * NSH
        deg_c = np.bincount(dst[core_of == c] - d0, minlength=NSH)
        order = np.argsort(-deg_c, kind="stable")
        pos_of[d0 + order] = np.arange(NSH)
        orders.append(np.concatenate([order + d0, np.full(NSHP - NSH, -1, np.int64)]))

    epos = pos_of[dst]
    etile = epos // 128
    esub = etile % NSUB

    # per-tile K = max degree (over cores)
    K = np.zeros(NTILES, np.int64)
    for c in range(C):
        deg_p = np.bincount(epos[core_of == c], minlength=NSHP)
        K = np.maximum(K, deg_p.reshape(NTILES, 128).max(1))

    # per-(core, sub) distinct-source renumbering; local row 0 = PAD
    loc = np.zeros((C, NSUB, N), np.int32)
    n_cs = np.zeros((C, NSUB), np.int64)
    for c in range(C):
        for s in range(NSUB):
            nodes = np.unique(src[(core_of == c) & (esub == s)])
            n_cs[c, s] = len(nodes)
            loc[c, s, nodes] = 1 + np.arange(len(nodes), dtype=np.int32)
    subrows = int(n_cs.max()) + 1
    SUBROWS = ((subrows + 1023) // 1024) * 1024
    assert SUBROWS <= 32768, f"sub-table too big: {SUBROWS}"

    # per-edge slot assignment: rank within (core, pos)
    okey = np.lexsort((epos, core_of))
    sc, pc, srt = core_of[okey], epos[okey], src[okey]
    gid = sc * NSHP + pc
    first = np.r_[True, gid[1:] != gid[:-1]]
    idx_lin = np.arange(len(gid))
    start = np.maximum.accumulate(np.where(first, idx_lin, 0))
    rank = idx_lin - start
    assert (rank < K[(pc // 128)]).all()

    plan = dict(core=sc, pos=pc, src=srt, slot=rank, K=K)

    # gather groups: consecutive tiles of one sub-shard, sum K <= budget
    groups = []           # list of (sub, [tiles])
    for s in range(NSUB):
        cur, acc = [], 0
        for t in range(s, NTILES, NSUB):
            k = int(K[t])
            if cur and acc + k > GROUP_SLOT_BUDGET:
                groups.append((s, cur))
                cur, acc = [], 0
            cur.append(t)
            acc += k
        if cur:
            groups.append((s, cur))

    return orders, pos_of, plan, groups, loc, n_cs, SUBROWS


def _wrap_idx(arr):
    """[K,128] slot-major idx array -> [128, 8K] wrapped+replicated int16."""
    flat = arr.reshape(-1)                       # i = k*128 + p
    w = flat.reshape(-1, 16).T                   # [16, NI/16]
    return np.tile(w, (8, 1)).astype(np.int16)


def _build_idx_tensor(plan, groups, loc):
    """Per-core [128, IDXCOLS] int16 idx tensor (local sub-table rows)."""
    K = plan["K"]
    ginfo = []            # (sub, off, KG, [(t, tile_off, k), ...])
    off = 0
    for (s, tl) in groups:
        KG = int(sum(K[t] for t in tl))
        tiles = []
        toff = 0
        for t in tl:
            tiles.append((t, toff, int(K[t])))
            toff += int(K[t])
        ginfo.append((s, off, KG, tiles))
        off += 8 * KG
    idxcols = off

    core_a, pos_a, src_a, slot_a = plan["core"], plan["pos"], plan["src"], plan["slot"]
    out = np.zeros((C, 128, idxcols), np.int16)
    for c in range(C):
        m = core_a == c
        pos, srcn, slot = pos_a[m], src_a[m], slot_a[m]
        tile = pos // 128
        part = pos % 128
        for (s, goff, KG, tiles) in ginfo:
            arr = np.zeros((KG, 128), np.int64)             # pad -> row 0
            lc = loc[c, s]
            for (t, toff, k) in tiles:
                tm = tile == t
                arr[toff + slot[tm], part[tm]] = lc[srcn[tm]]
            out[c, :, goff:goff + 8 * KG] = _wrap_idx(arr)
    return out, ginfo, idxcols


def _build_launch(kdim, ginfo, idxcols, SUBROWS):
    """One SPMD launch: Stage A (sub-tables) + Stage B (gather groups)."""
    import concourse.bacc as bacc
    import concourse.mybir as mybir
    from concourse.tile import TileContext

    f32 = mybir.dt.float32
    bf16 = mybir.dt.bfloat16
    TROWS = NSUB * SUBROWS
    nchunk_sub = SUBROWS // 128
    SCH = 8                        # chunks per PSUM bank / super-chunk
    nsuper_sub = (nchunk_sub + SCH - 1) // SCH

    nc = bacc.Bacc(None, target_bir_lowering=False, debug=True)
    hT = nc.declare_dram_parameter("hT", [kdim, TROWS], bf16, isOutput=False)
    hoT = nc.declare_dram_parameter("hoT", [kdim, NSHP], bf16, isOutput=False)
    wse = nc.declare_dram_parameter("wse", [kdim, 65], bf16, isOutput=False)
    rb = nc.declare_dram_parameter("rb", [128, 128], f32, isOutput=False)
    idx = nc.declare_dram_parameter("idx", [128, idxcols], mybir.dt.int16,
                                    isOutput=False)
    outp = nc.declare_dram_parameter("outp", [NSHP, 64], f32, isOutput=True)
    tabl = nc.dram_tensor("tabl", [TROWS, 64], f32)

    with TileContext(nc) as tc:
        with (
            tc.tile_pool(name="const", bufs=1) as cpool,
            tc.tile_pool(name="xin", bufs=3) as xin,
            tc.tile_pool(name="stage", bufs=3) as stage,
            tc.tile_pool(name="psA", bufs=3, space="PSUM") as psA,
            tc.tile_pool(name="psB", bufs=2, space="PSUM") as psB,
            tc.tile_pool(name="tg", bufs=2) as tgp,
            tc.tile_pool(name="pt", bufs=2) as ptp,
            tc.tile_pool(name="sm", bufs=3) as sm,
        ):
            wse_sb = cpool.tile([kdim, 65], bf16)
            nc.sync.dma_start(out=wse_sb[:, :], in_=wse[:, :])
            rb_sb = cpool.tile([128, 128], f32)
            nc.sync.dma_start(out=rb_sb[:, :], in_=rb[:, :])
            idx_sb = cpool.tile([128, idxcols], mybir.dt.int16)
            nc.sync.dma_start(out=idx_sb[:, :], in_=idx[:, :])
            ho_sb = cpool.tile([kdim, NSHP], bf16)
            nc.sync.dma_start(out=ho_sb[:, :], in_=hoT[:, :])
            ad_sb = cpool.tile([128, NTILES], f32)
            padrow = cpool.tile([128, 64], f32)
            nc.vector.memset(padrow[:, :], PAD_VAL)

            # ad for own (sorted) dsts: ho chunk . wse[:, 64]
            for t in range(NTILES):
                ps2 = psB.tile([128, 1], f32, tag="ps2")
                nc.tensor.matmul(ps2[:, :], ho_sb[:, 128 * t:128 * (t + 1)],
                                 wse_sb[:, 64:65], start=True, stop=True)
                nc.scalar.copy(ad_sb[:, t:t + 1], ps2[:, :])

            # Stage A: per sub-table, 8 chunks per PSUM bank, batched writes
            for s in range(NSUB):
                base = s * SUBROWS
                for sci in range(nsuper_sub):
                    c0 = sci * SCH
                    nch = min(SCH, nchunk_sub - c0)
                    cols = 128 * nch
                    xt = xin.tile([kdim, 1024], bf16, tag="xt")
                    nc.sync.dma_start(
                        out=xt[:, 0:cols],
                        in_=hT[:, base + 128 * c0:base + 128 * c0 + cols])
                    ps = psA.tile([128, 512], f32, tag="ps")
                    for j in range(nch):
                        nc.tensor.matmul(ps[:, 64 * j:64 * (j + 1)],
                                         xt[:, 128 * j:128 * (j + 1)],
                                         wse_sb[:, 0:64], start=True, stop=True)
                    st = stage.tile([128, 512], f32, tag="st")
                    nc.vector.tensor_copy(st[:, 0:64 * nch], ps[:, 0:64 * nch])
                    dst = tabl[base + 128 * c0:base + 128 * c0 + cols, :] \
                        .rearrange("(c p) f -> p c f", p=128)
                    nc.sync.dma_start(out=dst, in_=st[:, 0:64 * nch]
                                      .rearrange("p (c f) -> p c f", f=64))
                # pad row of this sub-table
                nc.sync.dma_start(out=tabl[base:base + 1, :], in_=padrow[0:1, :])

            # Stage B: gather groups
            for (s, goff, KG, tiles) in ginfo:
                base = s * SUBROWS
                ng = len(tiles)
                tg = tgp.tile([128, KG, 64], f32, tag="tg")
                nc.gpsimd.dma_gather(tg[:, :, :],
                                     tabl[base:base + SUBROWS, :],
                                     idx_sb[:, goff:goff + 8 * KG],
                                     128 * KG, 128 * KG, 64,
                                     single_packet=False)
                den_g = sm.tile([128, ng], f32, tag="den")
                p_list = []
                for i, (t, toff, k) in enumerate(tiles):
                    as_t = sm.tile([128, k], f32, tag=f"as{i}")
                    nc.vector.tensor_reduce(as_t[:, :],
                                            tg[:, toff:toff + k, :],
                                            mybir.AxisListType.X,
                                            mybir.AluOpType.add)
                    z_t = sm.tile([128, k], f32, tag=f"z{i}")
                    nc.vector.tensor_tensor(z_t[:, :], as_t[:, :],
                                            ad_sb[:, t:t + 1]
                                            .broadcast_to([128, k]),
                                            mybir.AluOpType.add)
                    s_t = sm.tile([128, k], f32, tag=f"s{i}")
                    nc.vector.scalar_tensor_tensor(s_t[:, :], z_t[:, :],
                                                   NEG_SLOPE, z_t[:, :],
                                                   mybir.AluOpType.mult,
                                                   mybir.AluOpType.max)
                    p_t = sm.tile([128, k], f32, tag=f"p{i}")
                    nc.scalar.activation(p_t[:, :], s_t[:, :],
                                         mybir.ActivationFunctionType.Exp,
                                         accum_out=den_g[:, i:i + 1])
                    p_list.append(p_t)
                rd_g = sm.tile([128, ng], f32, tag="rd")
                nc.vector.reciprocal(rd_g[:, :], den_g[:, :])
                for i, (t, toff, k) in enumerate(tiles):
                    p_t = p_list[i]
                    pt = ptp.tile([128, GROUP_SLOT_BUDGET, 64], f32, tag="pt")
                    p_b = p_t[:, :].unsqueeze(2).broadcast_to([128, k, 64])
                    nc.vector.tensor_tensor(pt[:, 0:k, :],
                                            tg[:, toff:toff + k, :], p_b,
                                            mybir.AluOpType.mult)
                    u = sm.tile([128, 64], f32, tag=f"u{i}")
                    nc.vector.tensor_reduce(u[:, :],
                                            pt[:, 0:k, :].transpose([0, 2, 1]),
                                            mybir.AxisListType.X,
                                            mybir.AluOpType.add)
                    o1 = sm.tile([128, 64], f32, tag=f"o1{i}")
                    nc.vector.scalar_tensor_tensor(o1[:, :], u[:, :],
                                                   rd_g[:, i:i + 1],
                                                   rb_sb[:, 0:64],
                                                   mybir.AluOpType.mult,
                                                   mybir.AluOpType.mult)
                    o2 = sm.tile([128, 64], f32, tag=f"o2{i}")
                    nc.vector.tensor_tensor(o2[:, :], o1[:, :],
                                            rb_sb[:, 64:128],
                                            mybir.AluOpType.add)
                    nc.sync.dma_start(out=outp[128 * t:128 * (t + 1), :],
                                      in_=o2[:, :])

    nc.compile()
    return nc


LAST = {}


def _assemble_hT(featT_bf, loc, n_cs, SUBROWS, kdim):
    """Per-core hT [kdim, NSUB*SUBROWS] bf16 with sub-table column layout."""
    hts = []
    for c in range(C):
        ht = np.zeros((kdim, NSUB * SUBROWS), featT_bf.dtype)
        for s in range(NSUB):
            n = int(n_cs[c, s])
            nodes = np.nonzero(loc[c, s])[0]
            # loc values are 1..n in node order (np.unique sorted)
            ht[:, s * SUBROWS + 1:s * SUBROWS + 1 + n] = featT_bf[:, nodes]
        hts.append(ht)
    return hts


def kernel(x, edge_index, W1, a_src1, a_dst1, b1, W2, a_src2, a_dst2, b2):
    from concourse.bass_utils import run_bass_kernel_spmd
    import ml_dtypes

    bf = np.dtype(ml_dtypes.bfloat16)

    x = np.asarray(x, np.float32)
    edge_index = np.asarray(edge_index)
    W1 = np.asarray(W1, np.float32); a_src1 = np.asarray(a_src1, np.float32)
    a_dst1 = np.asarray(a_dst1, np.float32); b1 = np.asarray(b1, np.float32)
    W2 = np.asarray(W2, np.float32); a_src2 = np.asarray(a_src2, np.float32)
    a_dst2 = np.asarray(a_dst2, np.float32); b2 = np.asarray(b2, np.float32)

    orders, pos_of, plan, groups, loc, n_cs, SUBROWS = _build_plan(edge_index)
    idx, ginfo, idxcols = _build_idx_tensor(plan, groups, loc)

    nc1 = _build_launch(IN, ginfo, idxcols, SUBROWS)
    nc2 = _build_launch(OUT, ginfo, idxcols, SUBROWS)

    def guard(a):
        return np.where(a == 0, np.float32(1e-30), a)

    w1se = np.concatenate([W1 * a_src1[None, :], (W1 @ a_dst1)[:, None]], 1).astype(bf)
    w2se = np.concatenate([W2 * a_src2[None, :], (W2 @ a_dst2)[:, None]], 1).astype(bf)
    rb1 = np.concatenate([np.tile(1.0 / guard(a_src1), (128, 1)),
                          np.tile(b1, (128, 1))], 1).astype(np.float32)
    rb2 = np.concatenate([np.tile(1.0 / guard(a_src2), (128, 1)),
                          np.tile(b2, (128, 1))], 1).astype(np.float32)

    # layer 1 inputs
    xT_bf = np.ascontiguousarray(x.T).astype(bf)            # [IN, N]
    hts1 = _assemble_hT(xT_bf, loc, n_cs, SUBROWS, IN)
    in_maps1 = []
    for c in range(C):
        own = orders[c]
        hoT = np.zeros((IN, NSHP), bf)
        real = own >= 0
        hoT[:, real] = xT_bf[:, own[real]]
        in_maps1.append({"hT": hts1[c], "hoT": hoT, "wse": w1se, "rb": rb1,
                        "idx": idx[c]})

    res1 = run_bass_kernel_spmd(nc1, in_maps1, core_ids=list(range(C)))
    LAST["res1"] = res1

    # h2 per node from pi-order shards
    h2 = np.zeros((N, OUT), np.float32)
    for c in range(C):
        sh = np.asarray(res1.results[c]["outp"])
        own = orders[c]
        real = own >= 0
        h2[own[real]] = sh[real]
    h2T_bf = np.ascontiguousarray(h2.T).astype(bf)          # [64, N]

    hts2 = _assemble_hT(h2T_bf, loc, n_cs, SUBROWS, OUT)
    in_maps2 = []
    for c in range(C):
        own = orders[c]
        hoT2 = np.zeros((OUT, NSHP), bf)
        real = own >= 0
        hoT2[:, real] = h2T_bf[:, own[real]]
        in_maps2.append({"hT": hts2[c], "hoT": hoT2, "wse": w2se, "rb": rb2,
                        "idx": idx[c]})

    res2 = run_bass_kernel_spmd(nc2, in_maps2, core_ids=list(range(C)))
    LAST["res2"] = res2

    out = np.empty((N, OUT), np.float32)
    for c in range(C):
        sh = np.asarray(res2.results[c]["outp"])
        own = orders[c]
        real = own >= 0
        out[own[real]] = sh[real]
    return out


# revision 12
# speedup vs baseline: 1.7470x; 1.2519x over previous
"""Two-layer single-head GAT (GATConv x2) on 8 trn2 NeuronCores.

Strategy: 1D node partition across 8 cores by destination node; edges live
with their destination owner, so edge-softmax and the scatter-aggregate stay
local. Weights replicated. Both layers share one graph plan / gather-index
tensor.

The bottleneck is SWDGE descriptor generation on gpsimd (~8ns/gathered row),
so the design minimizes gathered slots. dma_gather indices are int16, which
only reaches 32767 rows. Instead of splitting each destination's edges
across two gather windows (costs ~40% extra padded slots), each core's 49
destination tiles are split round-robin into 3 SUB-SHARDS; each sub-shard
gets its own COMPACT gather table holding just its distinct source nodes
(~27k rows < 32767), renumbered densely. Slot padding is then only the
per-tile max-degree padding (~2-3%).

Per layer, per core:
  Stage A (dense, PE, bf16): table rows T = h_src @ (W * a_src) written to
    the 3 concatenated sub-tables in DRAM (f32); host supplies hT with
    columns pre-arranged in sub-table order. ad = h_own @ (W @ a_dst).
    8 chunks per PSUM bank, batched 1024-row DMA writes.
  Stage B (sparse): destination tiles are degree-sorted, 128 dsts per tile
    (one per SBUF partition); tiles batched into gather GROUPS (sum K <= 96)
    within a sub-shard, ONE dma_gather per group. Per tile:
      as = rowsum(T_gathered)        (DVE reduce)
      s  = Lrelu(as + ad)            (Scalar activation, bias=ad)
      p  = Exp(s), den accumulated   (Scalar activation)
      rd = 1/den                     (DVE, batched per group)
      U  = sum_k p_k T_k             (DVE mult + transposed reduce)
      out = U * (1/a_src) * rd + b
  Padded slots point at each sub-table's row 0, filled with -1e30 => p == 0.
"""

import sys

sys.path.insert(0, "/opt/trn_rl_repo")

import numpy as np

N = 50000
E = 800000
IN = 128
OUT = 64
C = 8                       # cores
NSH = N // C                # 6250 dsts per core
NTILES = (NSH + 127) // 128  # 49
NSHP = NTILES * 128         # 6272 padded dsts per core
NEG_SLOPE = 0.2
NSUB = 4                    # sub-shards per core (tile t -> sub t % NSUB)
PAD_VAL = -1.0e30
GROUP_SLOT_BUDGET = 96      # max sum of K per gather group
LAST_SUB_BUDGET = 56        # smaller groups in the last sub-shard (short tail)


def _build_plan(edge_index):
    """Host-side graph preprocessing shared by both layers."""
    src = np.concatenate([np.asarray(edge_index[0], dtype=np.int64), np.arange(N)])
    dst = np.concatenate([np.asarray(edge_index[1], dtype=np.int64), np.arange(N)])

    core_of = dst // NSH
    orders = []
    pos_of = np.empty(N, dtype=np.int64)
    for c in range(C):
        d0 = c * NSH
        deg_c = np.bincount(dst[core_of == c] - d0, minlength=NSH)
        order = np.argsort(-deg_c, kind="stable")
        pos_of[d0 + order] = np.arange(NSH)
        orders.append(np.concatenate([order + d0, np.full(NSHP - NSH, -1, np.int64)]))

    epos = pos_of[dst]
    etile = epos // 128
    esub = etile % NSUB

    # per-tile K = max degree (over cores)
    K = np.zeros(NTILES, np.int64)
    for c in range(C):
        deg_p = np.bincount(epos[core_of == c], minlength=NSHP)
        K = np.maximum(K, deg_p.reshape(NTILES, 128).max(1))

    # per-(core, sub) distinct-source renumbering; local row 0 = PAD
    loc = np.zeros((C, NSUB, N), np.int32)
    n_cs = np.zeros((C, NSUB), np.int64)
    for c in range(C):
        for s in range(NSUB):
            nodes = np.unique(src[(core_of == c) & (esub == s)])
            n_cs[c, s] = len(nodes)
            loc[c, s, nodes] = 1 + np.arange(len(nodes), dtype=np.int32)
    subrows = int(n_cs.max()) + 1
    SUBROWS = ((subrows + 1023) // 1024) * 1024
    assert SUBROWS <= 32768, f"sub-table too big: {SUBROWS}"

    # per-edge slot assignment: rank within (core, pos)
    okey = np.lexsort((epos, core_of))
    sc, pc, srt = core_of[okey], epos[okey], src[okey]
    gid = sc * NSHP + pc
    first = np.r_[True, gid[1:] != gid[:-1]]
    idx_lin = np.arange(len(gid))
    start = np.maximum.accumulate(np.where(first, idx_lin, 0))
    rank = idx_lin - start
    assert (rank < K[(pc // 128)]).all()

    plan = dict(core=sc, pos=pc, src=srt, slot=rank, K=K)

    # gather groups: consecutive tiles of one sub-shard, sum K <= budget
    groups = []           # list of (sub, [tiles])
    for s in range(NSUB):
        budget = LAST_SUB_BUDGET if s == NSUB - 1 else GROUP_SLOT_BUDGET
        cur, acc = [], 0
        for t in range(s, NTILES, NSUB):
            k = int(K[t])
            if cur and acc + k > budget:
                groups.append((s, cur))
                cur, acc = [], 0
            cur.append(t)
            acc += k
        if cur:
            groups.append((s, cur))

    return orders, pos_of, plan, groups, loc, n_cs, SUBROWS


def _wrap_idx(arr):
    """[K,128] slot-major idx array -> [128, 8K] wrapped+replicated int16."""
    flat = arr.reshape(-1)                       # i = k*128 + p
    w = flat.reshape(-1, 16).T                   # [16, NI/16]
    return np.tile(w, (8, 1)).astype(np.int16)


def _build_idx_tensor(plan, groups, loc):
    """Per-core [128, IDXCOLS] int16 idx tensor (local sub-table rows)."""
    K = plan["K"]
    ginfo = []            # (sub, off, KG, [(t, tile_off, k), ...])
    off = 0
    for (s, tl) in groups:
        KG = int(sum(K[t] for t in tl))
        tiles = []
        toff = 0
        for t in tl:
            tiles.append((t, toff, int(K[t])))
            toff += int(K[t])
        ginfo.append((s, off, KG, tiles))
        off += 8 * KG
    idxcols = off

    core_a, pos_a, src_a, slot_a = plan["core"], plan["pos"], plan["src"], plan["slot"]
    out = np.zeros((C, 128, idxcols), np.int16)
    for c in range(C):
        m = core_a == c
        pos, srcn, slot = pos_a[m], src_a[m], slot_a[m]
        tile = pos // 128
        part = pos % 128
        for (s, goff, KG, tiles) in ginfo:
            arr = np.zeros((KG, 128), np.int64)             # pad -> row 0
            lc = loc[c, s]
            for (t, toff, k) in tiles:
                tm = tile == t
                arr[toff + slot[tm], part[tm]] = lc[srcn[tm]]
            out[c, :, goff:goff + 8 * KG] = _wrap_idx(arr)
    return out, ginfo, idxcols


def _build_launch(kdim, ginfo, idxcols, SUBROWS):
    """One SPMD launch: Stage A (sub-tables) + Stage B (gather groups)."""
    import concourse.bacc as bacc
    import concourse.mybir as mybir
    from concourse.tile import TileContext

    f32 = mybir.dt.float32
    bf16 = mybir.dt.bfloat16
    TROWS = NSUB * SUBROWS
    nchunk_sub = SUBROWS // 128
    SCH = 8                        # chunks per PSUM bank / super-chunk
    nsuper_sub = (nchunk_sub + SCH - 1) // SCH

    nc = bacc.Bacc(None, target_bir_lowering=False, debug=True)
    hT = nc.declare_dram_parameter("hT", [kdim, TROWS], bf16, isOutput=False)
    hoT = nc.declare_dram_parameter("hoT", [kdim, NSHP], bf16, isOutput=False)
    wse = nc.declare_dram_parameter("wse", [kdim, 65], bf16, isOutput=False)
    rb = nc.declare_dram_parameter("rb", [128, 128], f32, isOutput=False)
    idx = nc.declare_dram_parameter("idx", [128, idxcols], mybir.dt.int16,
                                    isOutput=False)
    outp = nc.declare_dram_parameter("outp", [NSHP, 64], f32, isOutput=True)
    tabl = nc.dram_tensor("tabl", [TROWS, 64], f32)

    with TileContext(nc) as tc:
        with (
            tc.tile_pool(name="const", bufs=1) as cpool,
            tc.tile_pool(name="xin", bufs=3) as xin,
            tc.tile_pool(name="stage", bufs=3) as stage,
            tc.tile_pool(name="psA", bufs=3, space="PSUM") as psA,
            tc.tile_pool(name="psB", bufs=2, space="PSUM") as psB,
            tc.tile_pool(name="tg", bufs=3) as tgp,
            tc.tile_pool(name="pt", bufs=2) as ptp,
            tc.tile_pool(name="sm", bufs=3) as sm,
        ):
            wse_sb = cpool.tile([kdim, 65], bf16)
            nc.sync.dma_start(out=wse_sb[:, :], in_=wse[:, :])
            rb_sb = cpool.tile([128, 128], f32)
            nc.sync.dma_start(out=rb_sb[:, :], in_=rb[:, :])
            idx_sb = cpool.tile([128, idxcols], mybir.dt.int16)
            nc.sync.dma_start(out=idx_sb[:, :], in_=idx[:, :])
            ho_sb = cpool.tile([kdim, NSHP], bf16)
            nc.sync.dma_start(out=ho_sb[:, :], in_=hoT[:, :])
            ad_sb = cpool.tile([128, NTILES], f32)
            padrow = cpool.tile([128, 64], f32)
            nc.vector.memset(padrow[:, :], PAD_VAL)

            # ad for own (sorted) dsts: ho chunk . wse[:, 64]
            for t in range(NTILES):
                ps2 = psB.tile([128, 1], f32, tag="ps2")
                nc.tensor.matmul(ps2[:, :], ho_sb[:, 128 * t:128 * (t + 1)],
                                 wse_sb[:, 64:65], start=True, stop=True)
                nc.scalar.copy(ad_sb[:, t:t + 1], ps2[:, :])

            # Stage A: per sub-table, 8 chunks per PSUM bank, batched writes
            for s in range(NSUB):
                base = s * SUBROWS
                for sci in range(nsuper_sub):
                    c0 = sci * SCH
                    nch = min(SCH, nchunk_sub - c0)
                    cols = 128 * nch
                    xt = xin.tile([kdim, 1024], bf16, tag="xt")
                    nc.sync.dma_start(
                        out=xt[:, 0:cols],
                        in_=hT[:, base + 128 * c0:base + 128 * c0 + cols])
                    ps = psA.tile([128, 512], f32, tag="ps")
                    for j in range(nch):
                        nc.tensor.matmul(ps[:, 64 * j:64 * (j + 1)],
                                         xt[:, 128 * j:128 * (j + 1)],
                                         wse_sb[:, 0:64], start=True, stop=True)
                    st = stage.tile([128, 512], f32, tag="st")
                    nc.vector.tensor_copy(st[:, 0:64 * nch], ps[:, 0:64 * nch])
                    dst = tabl[base + 128 * c0:base + 128 * c0 + cols, :] \
                        .rearrange("(c p) f -> p c f", p=128)
                    nc.sync.dma_start(out=dst, in_=st[:, 0:64 * nch]
                                      .rearrange("p (c f) -> p c f", f=64))
                # pad row of this sub-table
                nc.sync.dma_start(out=tabl[base:base + 1, :], in_=padrow[0:1, :])

            # Stage B: gather groups. Desc-gen (prepare_only) has no table
            # dependency — it runs from t=0, overlapped with Stage A; the
            # trigger carries the deferred table-read dep.
            for gi, (s, goff, KG, tiles) in enumerate(ginfo):
                base = s * SUBROWS
                ng = len(tiles)
                tg = tgp.tile([128, KG, 64], f32, tag="tg")
                dma_sem = nc.alloc_semaphore(f"swdge_g{gi}")
                nc.gpsimd.dma_gather(tg[:, :, :],
                                     tabl[base:base + SUBROWS, :],
                                     idx_sb[:, goff:goff + 8 * KG],
                                     128 * KG, 128 * KG, 64,
                                     single_packet=False,
                                     prepare_only=True, sem=dma_sem)
                nc.gpsimd.trigger_dma(count=None)
                # tg consumers are all Vector ops; the prep's tick only covers
                # desc-gen, so gate Vector on the DMA-completion sem itself.
                nc.vector.wait_ge(dma_sem, 16)
                den_g = sm.tile([128, ng], f32, tag="den")
                p_list = []
                for i, (t, toff, k) in enumerate(tiles):
                    as_t = sm.tile([128, k], f32, tag=f"as{i}")
                    nc.vector.tensor_reduce(as_t[:, :],
                                            tg[:, toff:toff + k, :],
                                            mybir.AxisListType.X,
                                            mybir.AluOpType.add)
                    z_t = sm.tile([128, k], f32, tag=f"z{i}")
                    nc.vector.tensor_tensor(z_t[:, :], as_t[:, :],
                                            ad_sb[:, t:t + 1]
                                            .broadcast_to([128, k]),
                                            mybir.AluOpType.add)
                    s_t = sm.tile([128, k], f32, tag=f"s{i}")
                    nc.vector.scalar_tensor_tensor(s_t[:, :], z_t[:, :],
                                                   NEG_SLOPE, z_t[:, :],
                                                   mybir.AluOpType.mult,
                                                   mybir.AluOpType.max)
                    p_t = sm.tile([128, k], f32, tag=f"p{i}")
                    nc.scalar.activation(p_t[:, :], s_t[:, :],
                                         mybir.ActivationFunctionType.Exp,
                                         accum_out=den_g[:, i:i + 1])
                    p_list.append(p_t)
                rd_g = sm.tile([128, ng], f32, tag="rd")
                nc.vector.reciprocal(rd_g[:, :], den_g[:, :])
                for i, (t, toff, k) in enumerate(tiles):
                    p_t = p_list[i]
                    pt = ptp.tile([128, GROUP_SLOT_BUDGET, 64], f32, tag="pt")
                    p_b = p_t[:, :].unsqueeze(2).broadcast_to([128, k, 64])
                    nc.vector.tensor_tensor(pt[:, 0:k, :],
                                            tg[:, toff:toff + k, :], p_b,
                                            mybir.AluOpType.mult)
                    u = sm.tile([128, 64], f32, tag=f"u{i}")
                    nc.vector.tensor_reduce(u[:, :],
                                            pt[:, 0:k, :].transpose([0, 2, 1]),
                                            mybir.AxisListType.X,
                                            mybir.AluOpType.add)
                    o1 = sm.tile([128, 64], f32, tag=f"o1{i}")
                    nc.vector.scalar_tensor_tensor(o1[:, :], u[:, :],
                                                   rd_g[:, i:i + 1],
                                                   rb_sb[:, 0:64],
                                                   mybir.AluOpType.mult,
                                                   mybir.AluOpType.mult)
                    o2 = sm.tile([128, 64], f32, tag=f"o2{i}")
                    nc.vector.tensor_tensor(o2[:, :], o1[:, :],
                                            rb_sb[:, 64:128],
                                            mybir.AluOpType.add)
                    nc.sync.dma_start(out=outp[128 * t:128 * (t + 1), :],
                                      in_=o2[:, :])

    nc.compile()
    return nc


LAST = {}


def _assemble_hT(featT_bf, loc, n_cs, SUBROWS, kdim):
    """Per-core hT [kdim, NSUB*SUBROWS] bf16 with sub-table column layout."""
    hts = []
    for c in range(C):
        ht = np.zeros((kdim, NSUB * SUBROWS), featT_bf.dtype)
        for s in range(NSUB):
            n = int(n_cs[c, s])
            nodes = np.nonzero(loc[c, s])[0]
            # loc values are 1..n in node order (np.unique sorted)
            ht[:, s * SUBROWS + 1:s * SUBROWS + 1 + n] = featT_bf[:, nodes]
        hts.append(ht)
    return hts


def kernel(x, edge_index, W1, a_src1, a_dst1, b1, W2, a_src2, a_dst2, b2):
    from concourse.bass_utils import run_bass_kernel_spmd
    import ml_dtypes

    bf = np.dtype(ml_dtypes.bfloat16)

    x = np.asarray(x, np.float32)
    edge_index = np.asarray(edge_index)
    W1 = np.asarray(W1, np.float32); a_src1 = np.asarray(a_src1, np.float32)
    a_dst1 = np.asarray(a_dst1, np.float32); b1 = np.asarray(b1, np.float32)
    W2 = np.asarray(W2, np.float32); a_src2 = np.asarray(a_src2, np.float32)
    a_dst2 = np.asarray(a_dst2, np.float32); b2 = np.asarray(b2, np.float32)

    orders, pos_of, plan, groups, loc, n_cs, SUBROWS = _build_plan(edge_index)
    idx, ginfo, idxcols = _build_idx_tensor(plan, groups, loc)

    nc1 = _build_launch(IN, ginfo, idxcols, SUBROWS)
    nc2 = _build_launch(OUT, ginfo, idxcols, SUBROWS)

    def guard(a):
        return np.where(a == 0, np.float32(1e-30), a)

    w1se = np.concatenate([W1 * a_src1[None, :], (W1 @ a_dst1)[:, None]], 1).astype(bf)
    w2se = np.concatenate([W2 * a_src2[None, :], (W2 @ a_dst2)[:, None]], 1).astype(bf)
    rb1 = np.concatenate([np.tile(1.0 / guard(a_src1), (128, 1)),
                          np.tile(b1, (128, 1))], 1).astype(np.float32)
    rb2 = np.concatenate([np.tile(1.0 / guard(a_src2), (128, 1)),
                          np.tile(b2, (128, 1))], 1).astype(np.float32)

    # layer 1 inputs
    xT_bf = np.ascontiguousarray(x.T).astype(bf)            # [IN, N]
    hts1 = _assemble_hT(xT_bf, loc, n_cs, SUBROWS, IN)
    in_maps1 = []
    for c in range(C):
        own = orders[c]
        hoT = np.zeros((IN, NSHP), bf)
        real = own >= 0
        hoT[:, real] = xT_bf[:, own[real]]
        in_maps1.append({"hT": hts1[c], "hoT": hoT, "wse": w1se, "rb": rb1,
                        "idx": idx[c]})

    res1 = run_bass_kernel_spmd(nc1, in_maps1, core_ids=list(range(C)))
    LAST["res1"] = res1

    # h2 per node from pi-order shards
    h2 = np.zeros((N, OUT), np.float32)
    for c in range(C):
        sh = np.asarray(res1.results[c]["outp"])
        own = orders[c]
        real = own >= 0
        h2[own[real]] = sh[real]
    h2T_bf = np.ascontiguousarray(h2.T).astype(bf)          # [64, N]

    hts2 = _assemble_hT(h2T_bf, loc, n_cs, SUBROWS, OUT)
    in_maps2 = []
    for c in range(C):
        own = orders[c]
        hoT2 = np.zeros((OUT, NSHP), bf)
        real = own >= 0
        hoT2[:, real] = h2T_bf[:, own[real]]
        in_maps2.append({"hT": hts2[c], "hoT": hoT2, "wse": w2se, "rb": rb2,
                        "idx": idx[c]})

    res2 = run_bass_kernel_spmd(nc2, in_maps2, core_ids=list(range(C)))
    LAST["res2"] = res2

    out = np.empty((N, OUT), np.float32)
    for c in range(C):
        sh = np.asarray(res2.results[c]["outp"])
        own = orders[c]
        real = own >= 0
        out[own[real]] = sh[real]
    return out


# revision 15
# speedup vs baseline: 1.8032x; 1.0322x over previous
"""Two-layer single-head GAT (GATConv x2) on 8 trn2 NeuronCores.

Strategy: 1D node partition across 8 cores by destination node; edges live
with their destination owner, so edge-softmax and the scatter-aggregate stay
local. Weights replicated. Both layers share one graph plan / gather-index
tensor.

The bottleneck is SWDGE descriptor generation on gpsimd (~8ns/gathered row),
so the design minimizes gathered slots. dma_gather indices are int16, which
only reaches 32767 rows. Instead of splitting each destination's edges
across two gather windows (costs ~40% extra padded slots), each core's 49
destination tiles are split round-robin into 3 SUB-SHARDS; each sub-shard
gets its own COMPACT gather table holding just its distinct source nodes
(~27k rows < 32767), renumbered densely. Slot padding is then only the
per-tile max-degree padding (~2-3%).

Per layer, per core:
  Stage A (dense, PE, bf16): table rows T = h_src @ (W * a_src) written to
    the 3 concatenated sub-tables in DRAM (f32); host supplies hT with
    columns pre-arranged in sub-table order. ad = h_own @ (W @ a_dst).
    8 chunks per PSUM bank, batched 1024-row DMA writes.
  Stage B (sparse): destination tiles are degree-sorted, 128 dsts per tile
    (one per SBUF partition); tiles batched into gather GROUPS (sum K <= 96)
    within a sub-shard, ONE dma_gather per group. Per tile:
      as = rowsum(T_gathered)        (DVE reduce)
      s  = Lrelu(as + ad)            (Scalar activation, bias=ad)
      p  = Exp(s), den accumulated   (Scalar activation)
      rd = 1/den                     (DVE, batched per group)
      U  = sum_k p_k T_k             (DVE mult + transposed reduce)
      out = U * (1/a_src) * rd + b
  Padded slots point at each sub-table's row 0, filled with -1e30 => p == 0.
"""

import sys

sys.path.insert(0, "/opt/trn_rl_repo")

import numpy as np

N = 50000
E = 800000
IN = 128
OUT = 64
C = 8                       # cores
NSH = N // C                # 6250 dsts per core
NTILES = (NSH + 127) // 128  # 49
NSHP = NTILES * 128         # 6272 padded dsts per core
NEG_SLOPE = 0.2
NSUB = 4                    # sub-shards per core (tile t -> sub t % NSUB)
PAD_VAL = -1.0e30
GROUP_SLOT_BUDGET = 96      # max sum of K per gather group
LAST_SUB_BUDGET = 56        # smaller groups in the last sub-shard (short tail)


def _build_plan(edge_index):
    """Host-side graph preprocessing shared by both layers."""
    src = np.concatenate([np.asarray(edge_index[0], dtype=np.int64), np.arange(N)])
    dst = np.concatenate([np.asarray(edge_index[1], dtype=np.int64), np.arange(N)])

    core_of = dst // NSH
    orders = []
    pos_of = np.empty(N, dtype=np.int64)
    for c in range(C):
        d0 = c * NSH
        deg_c = np.bincount(dst[core_of == c] - d0, minlength=NSH)
        order = np.argsort(-deg_c, kind="stable")
        pos_of[d0 + order] = np.arange(NSH)
        orders.append(np.concatenate([order + d0, np.full(NSHP - NSH, -1, np.int64)]))

    epos = pos_of[dst]
    etile = epos // 128
    esub = etile % NSUB

    # per-tile K = max degree (over cores)
    K = np.zeros(NTILES, np.int64)
    for c in range(C):
        deg_p = np.bincount(epos[core_of == c], minlength=NSHP)
        K = np.maximum(K, deg_p.reshape(NTILES, 128).max(1))

    # per-(core, sub) distinct-source renumbering; local row 0 = PAD
    loc = np.zeros((C, NSUB, N), np.int32)
    n_cs = np.zeros((C, NSUB), np.int64)
    for c in range(C):
        for s in range(NSUB):
            nodes = np.unique(src[(core_of == c) & (esub == s)])
            n_cs[c, s] = len(nodes)
            loc[c, s, nodes] = 1 + np.arange(len(nodes), dtype=np.int32)
    subrows = int(n_cs.max()) + 1
    SUBROWS = ((subrows + 1023) // 1024) * 1024
    assert SUBROWS <= 32768, f"sub-table too big: {SUBROWS}"

    # per-edge slot assignment: rank within (core, pos)
    okey = np.lexsort((epos, core_of))
    sc, pc, srt = core_of[okey], epos[okey], src[okey]
    gid = sc * NSHP + pc
    first = np.r_[True, gid[1:] != gid[:-1]]
    idx_lin = np.arange(len(gid))
    start = np.maximum.accumulate(np.where(first, idx_lin, 0))
    rank = idx_lin - start
    assert (rank < K[(pc // 128)]).all()

    plan = dict(core=sc, pos=pc, src=srt, slot=rank, K=K)

    # gather groups: consecutive tiles of one sub-shard, sum K <= budget
    groups = []           # list of (sub, [tiles])
    for s in range(NSUB):
        budget = LAST_SUB_BUDGET if s == NSUB - 1 else GROUP_SLOT_BUDGET
        cur, acc = [], 0
        for t in range(s, NTILES, NSUB):
            k = int(K[t])
            if cur and acc + k > budget:
                groups.append((s, cur))
                cur, acc = [], 0
            cur.append(t)
            acc += k
        if cur:
            groups.append((s, cur))

    return orders, pos_of, plan, groups, loc, n_cs, SUBROWS


def _wrap_idx(arr):
    """[K,128] slot-major idx array -> [128, 8K] wrapped+replicated int16."""
    flat = arr.reshape(-1)                       # i = k*128 + p
    w = flat.reshape(-1, 16).T                   # [16, NI/16]
    return np.tile(w, (8, 1)).astype(np.int16)


def _build_idx_tensor(plan, groups, loc):
    """Per-core [128, IDXCOLS] int16 idx tensor (local sub-table rows)."""
    K = plan["K"]
    ginfo = []            # (sub, off, KG, [(t, tile_off, k), ...])
    off = 0
    for (s, tl) in groups:
        KG = int(sum(K[t] for t in tl))
        tiles = []
        toff = 0
        for t in tl:
            tiles.append((t, toff, int(K[t])))
            toff += int(K[t])
        ginfo.append((s, off, KG, tiles))
        off += 8 * KG
    idxcols = off

    core_a, pos_a, src_a, slot_a = plan["core"], plan["pos"], plan["src"], plan["slot"]
    out = np.zeros((C, 128, idxcols), np.int16)
    for c in range(C):
        m = core_a == c
        pos, srcn, slot = pos_a[m], src_a[m], slot_a[m]
        tile = pos // 128
        part = pos % 128
        for (s, goff, KG, tiles) in ginfo:
            arr = np.zeros((KG, 128), np.int64)             # pad -> row 0
            lc = loc[c, s]
            for (t, toff, k) in tiles:
                tm = tile == t
                arr[toff + slot[tm], part[tm]] = lc[srcn[tm]]
            out[c, :, goff:goff + 8 * KG] = _wrap_idx(arr)
    return out, ginfo, idxcols


def _build_launch(kdim, ginfo, idxcols, SUBROWS):
    """One SPMD launch: Stage A (sub-tables) + Stage B (gather groups)."""
    import concourse.bacc as bacc
    import concourse.mybir as mybir
    from concourse.tile import TileContext

    f32 = mybir.dt.float32
    bf16 = mybir.dt.bfloat16
    TROWS = NSUB * SUBROWS
    nchunk_sub = SUBROWS // 128
    SCH = 8                        # chunks per PSUM bank / super-chunk
    nsuper_sub = (nchunk_sub + SCH - 1) // SCH

    nc = bacc.Bacc(None, target_bir_lowering=False, debug=True)
    hT = nc.declare_dram_parameter("hT", [kdim, TROWS], bf16, isOutput=False)
    hoT = nc.declare_dram_parameter("hoT", [kdim, NSHP], bf16, isOutput=False)
    wse = nc.declare_dram_parameter("wse", [kdim, 65], bf16, isOutput=False)
    rb = nc.declare_dram_parameter("rb", [128, 128], f32, isOutput=False)
    idx = nc.declare_dram_parameter("idx", [128, idxcols], mybir.dt.int16,
                                    isOutput=False)
    outp = nc.declare_dram_parameter("outp", [NSHP, 64], f32, isOutput=True)
    tabl = nc.dram_tensor("tabl", [TROWS, 64], f32)

    with TileContext(nc) as tc:
        with (
            tc.tile_pool(name="const", bufs=1) as cpool,
            tc.tile_pool(name="xin", bufs=3) as xin,
            tc.tile_pool(name="stage", bufs=3) as stage,
            tc.tile_pool(name="psA", bufs=3, space="PSUM") as psA,
            tc.tile_pool(name="psB", bufs=2, space="PSUM") as psB,
            tc.tile_pool(name="tg", bufs=4) as tgp,
            tc.tile_pool(name="pt", bufs=2) as ptp,
            tc.tile_pool(name="sm", bufs=3) as sm,
        ):
            # idx slices per sub-shard, sub-0 first: the first gather prep
            # only waits on its own slice, not the whole 1.7MB index upload.
            sub_idx_range = {}
            for (s, goff, KG, tiles) in ginfo:
                lo, hi = sub_idx_range.get(s, (goff, goff + 8 * KG))
                sub_idx_range[s] = (min(lo, goff), max(hi, goff + 8 * KG))
            idx_sb = cpool.tile([128, idxcols], mybir.dt.int16)
            for s in sorted(sub_idx_range):
                lo, hi = sub_idx_range[s]
                nc.sync.dma_start(out=idx_sb[:, lo:hi], in_=idx[:, lo:hi])
            wse_sb = cpool.tile([kdim, 65], bf16)
            nc.sync.dma_start(out=wse_sb[:, :], in_=wse[:, :])
            rb_sb = cpool.tile([128, 128], f32)
            nc.sync.dma_start(out=rb_sb[:, :], in_=rb[:, :])
            ho_sb = cpool.tile([kdim, NSHP], bf16)
            nc.sync.dma_start(out=ho_sb[:, :], in_=hoT[:, :])
            ad_sb = cpool.tile([128, NTILES], f32)
            padrow = cpool.tile([128, 64], f32)
            nc.vector.memset(padrow[:, :], PAD_VAL)

            # Stage A: per sub-table, 8 chunks per PSUM bank, batched
            # writes. The ad matmuls run after sub-0 so its gathers can
            # trigger as early as possible.
            for s in range(NSUB):
                if s == 1:
                    for t in range(NTILES):
                        ps2 = psB.tile([128, 1], f32, tag="ps2")
                        nc.tensor.matmul(ps2[:, :],
                                         ho_sb[:, 128 * t:128 * (t + 1)],
                                         wse_sb[:, 64:65], start=True,
                                         stop=True)
                        nc.scalar.copy(ad_sb[:, t:t + 1], ps2[:, :])
                base = s * SUBROWS
                for sci in range(nsuper_sub):
                    c0 = sci * SCH
                    nch = min(SCH, nchunk_sub - c0)
                    cols = 128 * nch
                    xt = xin.tile([kdim, 1024], bf16, tag="xt")
                    nc.sync.dma_start(
                        out=xt[:, 0:cols],
                        in_=hT[:, base + 128 * c0:base + 128 * c0 + cols])
                    ps = psA.tile([128, 512], f32, tag="ps")
                    for j in range(nch):
                        nc.tensor.matmul(ps[:, 64 * j:64 * (j + 1)],
                                         xt[:, 128 * j:128 * (j + 1)],
                                         wse_sb[:, 0:64], start=True, stop=True)
                    st = stage.tile([128, 512], f32, tag="st")
                    nc.vector.tensor_copy(st[:, 0:64 * nch], ps[:, 0:64 * nch])
                    dst = tabl[base + 128 * c0:base + 128 * c0 + cols, :] \
                        .rearrange("(c p) f -> p c f", p=128)
                    nc.sync.dma_start(out=dst, in_=st[:, 0:64 * nch]
                                      .rearrange("p (c f) -> p c f", f=64))
                # pad row of this sub-table
                nc.sync.dma_start(out=tabl[base:base + 1, :], in_=padrow[0:1, :])

            # Stage B: gather groups. Desc-gen (prepare_only) has no table
            # dependency — it runs from t=0, overlapped with Stage A. All of
            # a sub-shard's preps are emitted before its ONE trigger so the
            # trigger's table-read wait never stalls later desc-gen.
            def compute_group(tg, dma_sem, tiles):
                ng = len(tiles)
                # tg consumers are all Vector ops; the prep's tick only covers
                # desc-gen, so gate Vector on the DMA-completion sem itself.
                nc.vector.wait_ge(dma_sem, 16)
                den_g = sm.tile([128, ng], f32, tag="den")
                p_list = []
                for i, (t, toff, k) in enumerate(tiles):
                    as_t = sm.tile([128, k], f32, tag=f"as{i}")
                    nc.vector.tensor_reduce(as_t[:, :],
                                            tg[:, toff:toff + k, :],
                                            mybir.AxisListType.X,
                                            mybir.AluOpType.add)
                    z_t = sm.tile([128, k], f32, tag=f"z{i}")
                    nc.vector.tensor_tensor(z_t[:, :], as_t[:, :],
                                            ad_sb[:, t:t + 1]
                                            .broadcast_to([128, k]),
                                            mybir.AluOpType.add)
                    s_t = sm.tile([128, k], f32, tag=f"s{i}")
                    nc.vector.scalar_tensor_tensor(s_t[:, :], z_t[:, :],
                                                   NEG_SLOPE, z_t[:, :],
                                                   mybir.AluOpType.mult,
                                                   mybir.AluOpType.max)
                    p_t = sm.tile([128, k], f32, tag=f"p{i}")
                    nc.scalar.activation(p_t[:, :], s_t[:, :],
                                         mybir.ActivationFunctionType.Exp,
                                         accum_out=den_g[:, i:i + 1])
                    p_list.append(p_t)
                rd_g = sm.tile([128, ng], f32, tag="rd")
                nc.vector.reciprocal(rd_g[:, :], den_g[:, :])
                for i, (t, toff, k) in enumerate(tiles):
                    p_t = p_list[i]
                    pt = ptp.tile([128, k, 64], f32, tag="pt")
                    p_b = p_t[:, :].unsqueeze(2).broadcast_to([128, k, 64])
                    nc.vector.tensor_tensor(pt[:, 0:k, :],
                                            tg[:, toff:toff + k, :], p_b,
                                            mybir.AluOpType.mult)
                    u = sm.tile([128, 64], f32, tag=f"u{i}")
                    nc.vector.tensor_reduce(u[:, :],
                                            pt[:, 0:k, :].transpose([0, 2, 1]),
                                            mybir.AxisListType.X,
                                            mybir.AluOpType.add)
                    o1 = sm.tile([128, 64], f32, tag=f"o1{i}")
                    nc.vector.scalar_tensor_tensor(o1[:, :], u[:, :],
                                                   rd_g[:, i:i + 1],
                                                   rb_sb[:, 0:64],
                                                   mybir.AluOpType.mult,
                                                   mybir.AluOpType.mult)
                    o2 = sm.tile([128, 64], f32, tag=f"o2{i}")
                    nc.vector.tensor_tensor(o2[:, :], o1[:, :],
                                            rb_sb[:, 64:128],
                                            mybir.AluOpType.add)
                    nc.sync.dma_start(out=outp[128 * t:128 * (t + 1), :],
                                      in_=o2[:, :])

            for gi, (s, goff, KG, tiles) in enumerate(ginfo):
                base = s * SUBROWS
                tg = tgp.tile([128, KG, 64], f32, tag="tg")
                dma_sem = nc.alloc_semaphore(f"swdge_g{gi}")
                nc.gpsimd.dma_gather(tg[:, :, :],
                                     tabl[base:base + SUBROWS, :],
                                     idx_sb[:, goff:goff + 8 * KG],
                                     128 * KG, 128 * KG, 64,
                                     single_packet=False,
                                     prepare_only=True, sem=dma_sem)
                nc.gpsimd.trigger_dma(count=None)
                compute_group(tg, dma_sem, tiles)

    nc.compile()
    return nc


LAST = {}


def _assemble_hT(featT_bf, loc, n_cs, SUBROWS, kdim):
    """Per-core hT [kdim, NSUB*SUBROWS] bf16 with sub-table column layout."""
    hts = []
    for c in range(C):
        ht = np.zeros((kdim, NSUB * SUBROWS), featT_bf.dtype)
        for s in range(NSUB):
            n = int(n_cs[c, s])
            nodes = np.nonzero(loc[c, s])[0]
            # loc values are 1..n in node order (np.unique sorted)
            ht[:, s * SUBROWS + 1:s * SUBROWS + 1 + n] = featT_bf[:, nodes]
        hts.append(ht)
    return hts


def kernel(x, edge_index, W1, a_src1, a_dst1, b1, W2, a_src2, a_dst2, b2):
    from concourse.bass_utils import run_bass_kernel_spmd
    import ml_dtypes

    bf = np.dtype(ml_dtypes.bfloat16)

    x = np.asarray(x, np.float32)
    edge_index = np.asarray(edge_index)
    W1 = np.asarray(W1, np.float32); a_src1 = np.asarray(a_src1, np.float32)
    a_dst1 = np.asarray(a_dst1, np.float32); b1 = np.asarray(b1, np.float32)
    W2 = np.asarray(W2, np.float32); a_src2 = np.asarray(a_src2, np.float32)
    a_dst2 = np.asarray(a_dst2, np.float32); b2 = np.asarray(b2, np.float32)

    orders, pos_of, plan, groups, loc, n_cs, SUBROWS = _build_plan(edge_index)
    idx, ginfo, idxcols = _build_idx_tensor(plan, groups, loc)

    nc1 = _build_launch(IN, ginfo, idxcols, SUBROWS)
    nc2 = _build_launch(OUT, ginfo, idxcols, SUBROWS)

    def guard(a):
        return np.where(a == 0, np.float32(1e-30), a)

    w1se = np.concatenate([W1 * a_src1[None, :], (W1 @ a_dst1)[:, None]], 1).astype(bf)
    w2se = np.concatenate([W2 * a_src2[None, :], (W2 @ a_dst2)[:, None]], 1).astype(bf)
    rb1 = np.concatenate([np.tile(1.0 / guard(a_src1), (128, 1)),
                          np.tile(b1, (128, 1))], 1).astype(np.float32)
    rb2 = np.concatenate([np.tile(1.0 / guard(a_src2), (128, 1)),
                          np.tile(b2, (128, 1))], 1).astype(np.float32)

    # layer 1 inputs
    xT_bf = np.ascontiguousarray(x.T).astype(bf)            # [IN, N]
    hts1 = _assemble_hT(xT_bf, loc, n_cs, SUBROWS, IN)
    in_maps1 = []
    for c in range(C):
        own = orders[c]
        hoT = np.zeros((IN, NSHP), bf)
        real = own >= 0
        hoT[:, real] = xT_bf[:, own[real]]
        in_maps1.append({"hT": hts1[c], "hoT": hoT, "wse": w1se, "rb": rb1,
                        "idx": idx[c]})

    res1 = run_bass_kernel_spmd(nc1, in_maps1, core_ids=list(range(C)))
    LAST["res1"] = res1

    # h2 per node from pi-order shards
    h2 = np.zeros((N, OUT), np.float32)
    for c in range(C):
        sh = np.asarray(res1.results[c]["outp"])
        own = orders[c]
        real = own >= 0
        h2[own[real]] = sh[real]
    h2T_bf = np.ascontiguousarray(h2.T).astype(bf)          # [64, N]

    hts2 = _assemble_hT(h2T_bf, loc, n_cs, SUBROWS, OUT)
    in_maps2 = []
    for c in range(C):
        own = orders[c]
        hoT2 = np.zeros((OUT, NSHP), bf)
        real = own >= 0
        hoT2[:, real] = h2T_bf[:, own[real]]
        in_maps2.append({"hT": hts2[c], "hoT": hoT2, "wse": w2se, "rb": rb2,
                        "idx": idx[c]})

    res2 = run_bass_kernel_spmd(nc2, in_maps2, core_ids=list(range(C)))
    LAST["res2"] = res2

    out = np.empty((N, OUT), np.float32)
    for c in range(C):
        sh = np.asarray(res2.results[c]["outp"])
        own = orders[c]
        real = own >= 0
        out[own[real]] = sh[real]
    return out
